# revision 3
# baseline (speedup 1.0000x reference)
"""BiLSTM layer (B=32, T=512, D=512, H=512) as a Bass/Trainium2 kernel on 8 NeuronCores.

Sharding: 8 cores = 2 directions x 4 batch-quarters. Each core runs a full
forward LSTM scan over T=512 steps for 8 examples of one direction. The
backward direction is realized as a forward scan over per-example
reversed-and-left-aligned inputs prepared on the host:
    x_rev[b, s] = x[b, len_b-1-s]  for s < len_b, else 0.
With that alignment no masking is needed anywhere:
  - forward: the scan past len_b computes garbage, but the host replaces the
    padded tail with h[len-1] via a gather (the reference freezes h there).
  - backward: the scan over s < len_b is exactly the reference's reversed
    scan; outputs are re-scattered per example on the host, zeros elsewhere.

Gate pre-activations live directly in PSUM: a windowed prepass (W=16 steps)
computes xg = W_ih.T x + (b_ih + b_hh) into the per-step gate PSUM columns
(bias enters via a 5th matmul against a ones-row), and the per-step
recurrent matmuls accumulate W_hh.T h on top (start=False). This removes
the per-step DVE gate-add and all Act-engine prepass work.

The tanh for the g-gate is computed as 2*sigmoid(2z)-1 (the 2z folding is
pre-baked into the g rows/bias), so the i/f/g gates need one sigmoid per
step; tanh(c) and sigmoid(o) are separate Act ops.
"""

import os
import sys

import numpy as np

sys.path.insert(0, "/opt/trn_rl_repo")

import concourse.bass as bass  # noqa: E402
import concourse.bacc as bacc  # noqa: E402
import concourse.tile as tile  # noqa: E402
from concourse import mybir  # noqa: E402

import ml_dtypes  # noqa: E402

F32 = mybir.dt.float32
F16 = mybir.dt.float16
F8 = mybir.dt.float8e4  # e4m3 — used for W_hh only
F8_NP = mybir.dt.np(F8)
AF = mybir.ActivationFunctionType
ALU = mybir.AluOpType

B, D, H = 32, 512, 512
G = 4 * H  # 2048 gate channels
NCORES = 8
BC = 8  # batch per core
KT = D // 128  # 4 k-tiles
MT = G // 128  # 16 m-tiles
WIN = 16  # steps per gate-PSUM window (16 m-tiles * WIN*BC f32 = 8KB/partition)

_T_DEFAULT = 512


def _build_nc(T: int, variant: str = "v2"):
    """Build the SPMD single-core program (identical on all 8 cores)."""
    nwin = T // WIN
    nc = bacc.Bacc("TRN2", target_bir_lowering=False, debug=False, num_devices=NCORES)

    xT_d = nc.dram_tensor("xT", [D, T * BC], F16, kind="ExternalInput")
    wih_d = nc.dram_tensor("wih", [D, G], F16, kind="ExternalInput")
    whh_d = nc.dram_tensor("whh", [H, G], F8, kind="ExternalInput")
    wb_d = nc.dram_tensor("wb", [128, G], F16, kind="ExternalInput")
    ones_d = nc.dram_tensor("ones", [128, WIN * BC], F16, kind="ExternalInput")
    hout_d = nc.dram_tensor(
        "hout", [nwin, 128, WIN * KT * BC], F16, kind="ExternalOutput"
    )

    with tile.TileContext(nc) as tc:
        with (
            tc.tile_pool(name="const", bufs=1) as constp,
            tc.tile_pool(name="xc", bufs=2) as xcp,
            tc.tile_pool(name="hw", bufs=2) as hp,
            tc.tile_pool(name="state_c", bufs=2) as cp,
            tc.tile_pool(name="sig", bufs=2) as sgp,
            tc.tile_pool(name="sigo", bufs=2) as sop,
            tc.tile_pool(name="ew", bufs=2) as ewp,
            tc.tile_pool(name="gw", bufs=2, space="PSUM") as gp,
            tc.tile_pool(name="gwo", bufs=2, space="PSUM") as gop,
        ):
            # ---- persistent weights in SBUF ----
            wih_sb = constp.tile([128, KT, G], F16, tag="wih")
            whh_sb = constp.tile([128, KT, G], F8, tag="whh")
            wb_sb = constp.tile([128, G], F16, tag="wb")
            ones_sb = constp.tile([128, WIN * BC], F16, tag="ones")
            for k in range(KT):
                nc.sync.dma_start(wih_sb[:, k, :], wih_d[k * 128:(k + 1) * 128, :])
                nc.sync.dma_start(whh_sb[:, k, :], whh_d[k * 128:(k + 1) * 128, :])
            nc.sync.dma_start(wb_sb[:], wb_d[:])
            nc.sync.dma_start(ones_sb[:], ones_d[:])

            # ---- initial state ----
            h0 = constp.tile([128, KT, BC], F16, tag="h0")
            nc.vector.memset(h0[:], 0.0)
            c_prev = cp.tile([128, KT, BC], F32, tag="c")
            nc.vector.memset(c_prev[:], 0.0)

            gw_tiles = {}

            # ---- prepass: xg (+bias) for one window, directly into PSUM ----
            def make_prepass(w):
                gw = gp.tile([128, MT, WIN * BC], F32, tag="gw")
                xc = xcp.tile([128, KT, WIN * BC], F16, tag="xc")
                gw_tiles[w] = gw
                c0 = w * WIN * BC
                chunks = []

                def dma_in():
                    for k in range(KT):
                        nc.sync.dma_start(
                            xc[:, k, :], xT_d[k * 128:(k + 1) * 128, c0:c0 + WIN * BC]
                        )

                chunks.append(dma_in)

                def mtile(m):
                    # 4 m-slices (512B each) share a 2KB PSUM zero region; the
                    # first matmul into each region uses start=True (marks the
                    # whole region pending-zero), the rest write/accumulate.
                    first = m % 4 == 0
                    for k in range(KT):
                        nc.tensor.matmul(
                            gw[:, m, :],
                            wih_sb[:, k, m * 128:(m + 1) * 128],
                            xc[:, k, :],
                            start=(first and k == 0),
                            stop=False,
                            skip_group_check=True,
                        )
                    # bias via ones-row: out[p, col] += wb[0, p] (row 0 of ones = 1)
                    nc.tensor.matmul(
                        gw[:, m, :],
                        wb_sb[:, m * 128:(m + 1) * 128],
                        ones_sb[:],
                        start=False,
                        stop=False,
                        skip_group_check=True,
                    )

                for m in range(MT):
                    chunks.append(lambda m=m: mtile(m))
                return chunks

            # prologue: window 0 fully
            for fn in make_prepass(0):
                fn()

            h_prev = h0[:]
            pending = []
            n_issued = 0
            hwt = None
            for t in range(T):
                w, s = divmod(t, WIN)
                if s == 0:
                    assert not pending, f"window {w}: {len(pending)} chunks undrained"
                    if w + 1 < nwin:
                        pending = make_prepass(w + 1)
                        n_issued = 0
                    hwt = hp.tile([128, WIN, KT, BC], F16, tag="hw")
                gw = gw_tiles[w]
                col = slice(s * BC, (s + 1) * BC)

                # recurrent matmuls accumulate onto the prepass xg columns
                for m in range(12):
                    for k in range(KT):
                        nc.tensor.matmul(
                            gw[:, m, col],
                            whh_sb[:, k, m * 128:(m + 1) * 128],
                            h_prev[:, k, :],
                            start=False,
                            stop=(k == KT - 1),
                            skip_group_check=True,
                        )
                sig = sgp.tile([128, 12, BC], F32, tag="sig")
                nc.scalar.activation(sig[:], gw[:, 0:12, col], AF.Sigmoid)
                for m in range(12, MT):
                    for k in range(KT):
                        nc.tensor.matmul(
                            gw[:, m, col],
                            whh_sb[:, k, m * 128:(m + 1) * 128],
                            h_prev[:, k, :],
                            start=False,
                            stop=(k == KT - 1),
                            skip_group_check=True,
                        )
                so = sop.tile([128, 4, BC], F32, tag="sigo")
                nc.scalar.activation(so[:], gw[:, 12:16, col], AF.Sigmoid)

                # c path: c_new = f*c + i*(2*sigmoid(2 z_g) - 1)
                fc = ewp.tile([128, KT, BC], F32, tag="fc")
                nc.vector.tensor_mul(fc[:], sig[:, 4:8, :], c_prev[:])
                t1 = ewp.tile([128, KT, BC], F32, tag="t1")
                nc.vector.scalar_tensor_tensor(
                    t1[:], sig[:, 8:12, :], 0.5, sig[:, 0:4, :],
                    ALU.subtract, ALU.mult,
                )
                c_new = cp.tile([128, KT, BC], F32, tag="c")
                nc.vector.scalar_tensor_tensor(
                    c_new[:], t1[:], 2.0, fc[:], ALU.mult, ALU.add
                )
                tct = ewp.tile([128, KT, BC], F32, tag="tct")
                nc.scalar.activation(tct[:], c_new[:], AF.Tanh)
                nc.vector.tensor_mul(hwt[:, s, :, :], so[:], tct[:])

                h_prev = hwt[:, s, :, :]
                c_prev = c_new

                # drip next window's prepass (17 chunks over WIN steps)
                if pending:
                    want = (s + 1) * 17 // WIN
                    while n_issued < want and pending:
                        pending.pop(0)()
                        n_issued += 1

                if s == WIN - 1:
                    nc.sync.dma_start(
                        hout_d[w], hwt[:].rearrange("p s k b -> p (s k b)")
                    )

    nc.compile()
    return nc


_NC_CACHE = {}


def _get_nc(T, variant=None):
    variant = variant or os.environ.get("BASS_LSTM_VARIANT", "v2")
    key = (T, variant)
    if key not in _NC_CACHE:
        _NC_CACHE[key] = _build_nc(T, variant)
    return _NC_CACHE[key]


_RUNNER_CACHE = {}


def _get_runner(nc):
    """Compile the SPMD executable once per program; reuse across calls.

    Forked from concourse.bass2jax.run_bass_via_pjrt (the @via_axon
    redirect target), minus the NTFF-trace path (unavailable here) and
    with the jitted callable cached so repeat kernel() calls skip the
    multi-minute walrus compile.
    """
    if id(nc) in _RUNNER_CACHE:
        return _RUNNER_CACHE[id(nc)]
    import jax
    from jax.sharding import Mesh, PartitionSpec
    from jax.experimental.shard_map import shard_map
    from concourse import bass2jax

    bass2jax.install_neuronx_cc_hook()

    partition_name = (
        nc.partition_id_tensor.name if nc.partition_id_tensor is not None else None
    )
    in_names, out_names, out_avals, zero_shapes = [], [], [], []
    for alloc in nc.m.functions[0].allocations:
        if not isinstance(alloc, mybir.MemoryLocationSet):
            continue
        name = alloc.memorylocations[0].name
        if alloc.kind == "ExternalInput":
            if name != partition_name:
                in_names.append(name)
        elif alloc.kind == "ExternalOutput":
            shape = tuple(alloc.tensor_shape)
            dtype = mybir.dt.np(alloc.dtype)
            out_names.append(name)
            out_avals.append(jax.core.ShapedArray(shape, dtype))
            zero_shapes.append((shape, dtype))
    n_params = len(in_names)
    all_in_names = in_names + out_names
    if partition_name is not None:
        all_in_names = all_in_names + [partition_name]

    def _body(*args):
        operands = list(args)
        if partition_name is not None:
            operands.append(bass2jax.partition_id_tensor())
        outs = bass2jax._bass_exec_p.bind(
            *operands,
            out_avals=tuple(out_avals),
            in_names=tuple(all_in_names),
            out_names=tuple(out_names),
            lowering_input_output_aliases=(),
            sim_require_finite=True,
            sim_require_nnan=True,
            nc=nc,
        )
        return tuple(outs)

    devices = jax.devices()[:NCORES]
    mesh = Mesh(np.asarray(devices), ("core",))
    nspecs = n_params + len(out_names)
    sharded = jax.jit(
        shard_map(
            _body,
            mesh=mesh,
            in_specs=(PartitionSpec("core"),) * nspecs,
            out_specs=(PartitionSpec("core"),) * len(out_names),
            check_rep=False,
        ),
        donate_argnums=tuple(range(n_params, nspecs)),
        keep_unused=True,
    )
    runner = (sharded, in_names, out_names, out_avals, zero_shapes)
    _RUNNER_CACHE[id(nc)] = runner
    return runner


def _run_spmd(nc, in_maps):
    sharded, in_names, out_names, out_avals, zero_shapes = _get_runner(nc)
    concat_in = [
        np.concatenate([np.asarray(in_maps[c][name]) for c in range(NCORES)], axis=0)
        for name in in_names
    ]
    concat_zeros = [
        np.zeros((NCORES * s[0], *s[1:]), dt) for (s, dt) in zero_shapes
    ]
    import time as _time

    t0 = _time.perf_counter()
    out_arrs = sharded(*concat_in, *concat_zeros)
    out_arrs = [np.asarray(a) for a in out_arrs]
    _run_spmd.last_wall_s = _time.perf_counter() - t0
    return [
        {
            name: out_arrs[i].reshape(NCORES, *out_avals[i].shape)[c]
            for i, name in enumerate(out_names)
        }
        for c in range(NCORES)
    ]


_run_spmd.last_wall_s = None


def _prep_core_inputs(x, lengths, wih, whh, wb, q, reverse, T):
    """Host-side input prep for one core (batch quarter q, one direction)."""
    xs = x[q * BC:(q + 1) * BC, :, :]  # [BC, T, D]
    ls = lengths[q * BC:(q + 1) * BC]  # [BC]
    if reverse:
        # per-example reversed-and-left-aligned: x_rev[b, s] = x[b, len-1-s]
        idx = ls[:, None] - 1 - np.arange(T)[None, :]  # [BC, T]
        valid = idx >= 0
        idx = np.maximum(idx, 0)
        xs = xs[np.arange(BC)[:, None], idx] * valid[:, :, None]
    xT = np.ascontiguousarray(xs.transpose(2, 1, 0).reshape(D, T * BC))
    ones = np.zeros((128, WIN * BC), np.float16)
    ones[0, :] = 1.0
    return {
        "xT": xT.astype(np.float16),
        "wih": wih,
        "whh": whh,
        "wb": wb,
        "ones": ones,
    }


def _prep_direction_weights(W_ih, W_hh, b_ih, b_hh):
    wih = np.ascontiguousarray(W_ih.T).astype(np.float32).copy()  # [D, G]
    whh = np.ascontiguousarray(W_hh.T).astype(np.float32).copy()  # [H, G]
    bsum = (b_ih + b_hh).astype(np.float32).copy()  # [G]
    # fold the tanh-via-sigmoid 2x into the g-gate block (cols 2H:3H)
    wih[:, 2 * H:3 * H] *= 2.0
    whh[:, 2 * H:3 * H] *= 2.0
    bsum[2 * H:3 * H] *= 2.0
    wb = np.zeros((128, G), np.float32)
    wb[0, :] = bsum
    return (
        wih.astype(np.float16),
        whh.astype(F8_NP),
        wb.astype(np.float16),
    )


def _scan_from_hout(hout, T):
    """hout [nwin, 128, WIN*KT*BC] -> scan output [BC, T, H] f32."""
    nwin = T // WIN
    a = np.asarray(hout).astype(np.float32).reshape(nwin, 128, WIN, KT, BC)
    # h[b, w*WIN+s, kt*128+p] = a[w, p, s, kt, b]
    return a.transpose(4, 0, 2, 3, 1).reshape(BC, T, H)


def kernel(x, lengths, W_ih_f, W_hh_f, b_ih_f, b_hh_f, W_ih_b, W_hh_b, b_ih_b, b_hh_b):
    T = x.shape[1]
    x = np.asarray(x, dtype=np.float32)
    lengths = np.asarray(lengths).astype(np.int64)

    wih_f, whh_f, wb_f = _prep_direction_weights(W_ih_f, W_hh_f, b_ih_f, b_hh_f)
    wih_b, whh_b, wb_b = _prep_direction_weights(W_ih_b, W_hh_b, b_ih_b, b_hh_b)

    in_maps = []
    for r in range(NCORES):
        reverse = r >= 4
        q = r % 4
        if reverse:
            m = _prep_core_inputs(x, lengths, wih_b, whh_b, wb_b, q, True, T)
        else:
            m = _prep_core_inputs(x, lengths, wih_f, whh_f, wb_f, q, False, T)
        in_maps.append(m)

    nc = _get_nc(T)
    results = _run_spmd(nc, in_maps)
    kernel.last_wall_s = _run_spmd.last_wall_s

    h_f = np.empty((B, T, H), np.float32)
    h_b = np.empty((B, T, H), np.float32)
    tidx = np.arange(T)
    for q in range(4):
        ls = lengths[q * BC:(q + 1) * BC]
        # forward: replace padded tail with h[len-1]
        hs = _scan_from_hout(results[q]["hout"], T)  # [BC, T, H]
        idx = np.minimum(tidx[None, :], (ls - 1)[:, None])  # [BC, T]
        h_f[q * BC:(q + 1) * BC] = hs[np.arange(BC)[:, None], idx]
        # backward: h_b[b, t] = h_scan[b, len-1-t] for t < len else 0
        hs = _scan_from_hout(results[q + 4]["hout"], T)
        idx = ls[:, None] - 1 - tidx[None, :]
        valid = idx >= 0
        idx = np.maximum(idx, 0)
        h_b[q * BC:(q + 1) * BC] = hs[np.arange(BC)[:, None], idx] * valid[:, :, None]

    return np.concatenate([h_f, h_b], axis=-1).astype(np.float32)


kernel.last_exec_time_ns = None
kernel.last_wall_s = None


# revision 7
# speedup vs baseline: 1.2440x; 1.2440x over previous
"""BiLSTM layer (B=32, T=512, D=512, H=512) as a Bass/Trainium2 kernel on 8 NeuronCores.

Sharding: 8 cores = 2 directions x 4 batch-quarters. Each core runs a full
forward LSTM scan over T=512 steps for 8 examples of one direction. The
backward direction is realized as a forward scan over per-example
reversed-and-left-aligned inputs prepared on the host:
    x_rev[b, s] = x[b, len_b-1-s]  for s < len_b, else 0.
With that alignment no masking is needed anywhere:
  - forward: the scan past len_b computes garbage, but the host replaces the
    padded tail with h[len-1] via a gather (the reference freezes h there).
  - backward: the scan over s < len_b is exactly the reference's reversed
    scan; outputs are re-scattered per example on the host, zeros elsewhere.

Gate pre-activations live directly in PSUM: a windowed prepass (W=16 steps)
computes xg = W_ih.T x + (b_ih + b_hh) into the per-step gate PSUM columns
(bias enters via a 5th matmul against a ones-row), and the per-step
recurrent matmuls accumulate W_hh.T h on top (start=False). This removes
the per-step DVE gate-add and all Act-engine prepass work.

The tanh for the g-gate is computed as 2*sigmoid(2z)-1 (the 2z folding is
pre-baked into the g rows/bias), so the i/f/g gates need one sigmoid per
step; tanh(c) and sigmoid(o) are separate Act ops.
"""

import os
import sys

import numpy as np

sys.path.insert(0, "/opt/trn_rl_repo")

import concourse.bass as bass  # noqa: E402
import concourse.bacc as bacc  # noqa: E402
import concourse.tile as tile  # noqa: E402
from concourse import mybir  # noqa: E402

import ml_dtypes  # noqa: E402

F32 = mybir.dt.float32
F16 = mybir.dt.float16
F8 = mybir.dt.float8e4  # e4m3 — used for W_hh only
F8_NP = mybir.dt.np(F8)
AF = mybir.ActivationFunctionType
ALU = mybir.AluOpType

B, D, H = 32, 512, 512
G = 4 * H  # 2048 gate channels
NCORES = 8
BC = 8  # batch per core
KT = D // 128  # 4 k-tiles
MT = G // 128  # 16 m-tiles
WIN = 16  # steps per gate-PSUM window (16 m-tiles * WIN*BC f32 = 8KB/partition)

_T_DEFAULT = 512


def _build_nc(T: int, variant: str = "v2"):
    """Build the SPMD single-core program (identical on all 8 cores)."""
    nwin = T // WIN
    nc = bacc.Bacc("TRN2", target_bir_lowering=False, debug=False, num_devices=NCORES)

    xT_d = nc.dram_tensor("xT", [D, T * BC], F16, kind="ExternalInput")
    wih_d = nc.dram_tensor("wih", [D, G], F16, kind="ExternalInput")
    whh_d = nc.dram_tensor("whh", [H, G], F8, kind="ExternalInput")
    wb_d = nc.dram_tensor("wb", [128, G], F16, kind="ExternalInput")
    ones_d = nc.dram_tensor("ones", [128, WIN * BC], F16, kind="ExternalInput")
    hout_d = nc.dram_tensor(
        "hout", [nwin, 128, WIN * KT * BC], F16, kind="ExternalOutput"
    )

    with tile.TileContext(nc) as tc:
        with (
            tc.tile_pool(name="const", bufs=1) as constp,
            tc.tile_pool(name="xc", bufs=2) as xcp,
            tc.tile_pool(name="hw", bufs=2) as hp,
            tc.tile_pool(name="state_c", bufs=3) as cp,
            tc.tile_pool(name="sig", bufs=3) as sgp,
            tc.tile_pool(name="sigo", bufs=3) as sop,
            tc.tile_pool(name="ew", bufs=3) as ewp,
            tc.tile_pool(name="gw", bufs=2, space="PSUM") as gp,
            tc.tile_pool(name="gwo", bufs=2, space="PSUM") as gop,
        ):
            # ---- persistent weights in SBUF ----
            wih_sb = constp.tile([128, KT, G], F16, tag="wih")
            whh_sb = constp.tile([128, KT, G], F8, tag="whh")
            wb_sb = constp.tile([128, G], F16, tag="wb")
            ones_sb = constp.tile([128, WIN * BC], F16, tag="ones")
            for k in range(KT):
                nc.sync.dma_start(wih_sb[:, k, :], wih_d[k * 128:(k + 1) * 128, :])
                nc.sync.dma_start(whh_sb[:, k, :], whh_d[k * 128:(k + 1) * 128, :])
            nc.sync.dma_start(wb_sb[:], wb_d[:])
            nc.sync.dma_start(ones_sb[:], ones_d[:])

            # ---- initial state ----
            h0 = constp.tile([128, KT, BC], F16, tag="h0")
            nc.vector.memset(h0[:], 0.0)
            c_prev = cp.tile([128, KT, BC], F32, tag="c")
            nc.vector.memset(c_prev[:], 0.0)

            gw_tiles = {}

            # ---- prepass: xg (+bias) for one window, directly into PSUM ----
            def make_prepass(w):
                # ifg gates (m 0..11) and o gates (m 12..15) in separate PSUM
                # tiles so the per-step o-gate matmuls don't pick up a false
                # WAR against sigma_ifg's read of the shared tile.
                gw = gp.tile([128, 12, WIN * BC], F32, tag="gw")
                gwo = gop.tile([128, 4, WIN * BC], F32, tag="gwo")
                xc = xcp.tile([128, KT, WIN * BC], F16, tag="xc")
                gw_tiles[w] = (gw, gwo)
                c0 = w * WIN * BC
                chunks = []

                def dma_in():
                    for k in range(KT):
                        nc.sync.dma_start(
                            xc[:, k, :], xT_d[k * 128:(k + 1) * 128, c0:c0 + WIN * BC]
                        )

                chunks.append(dma_in)

                def mtile(m):
                    # 4 m-slices (512B each) share a 2KB PSUM zero region; the
                    # first matmul into each region uses start=True (marks the
                    # whole region pending-zero), the rest write/accumulate.
                    dst = gw[:, m, :] if m < 12 else gwo[:, m - 12, :]
                    first = m % 4 == 0
                    for k in range(KT):
                        nc.tensor.matmul(
                            dst,
                            wih_sb[:, k, m * 128:(m + 1) * 128],
                            xc[:, k, :],
                            start=(first and k == 0),
                            stop=False,
                            skip_group_check=True,
                        )
                    # bias via ones-row: out[p, col] += wb[0, p] (row 0 of ones = 1)
                    nc.tensor.matmul(
                        dst,
                        wb_sb[:, m * 128:(m + 1) * 128],
                        ones_sb[:],
                        start=False,
                        stop=False,
                        skip_group_check=True,
                    )

                for m in range(MT):
                    chunks.append(lambda m=m: mtile(m))
                return chunks

            # prologue: window 0 fully
            for fn in make_prepass(0):
                fn()

            h_prev = h0[:]
            pending = []
            n_issued = 0
            hwt = None
            for t in range(T):
                w, s = divmod(t, WIN)
                if s == 0:
                    assert not pending, f"window {w}: {len(pending)} chunks undrained"
                    if w + 1 < nwin:
                        pending = make_prepass(w + 1)
                        n_issued = 0
                    hwt = hp.tile([128, WIN, KT, BC], F16, tag="hw")
                gw, gwo = gw_tiles[w]
                col = slice(s * BC, (s + 1) * BC)

                # recurrent matmuls accumulate onto the prepass xg columns
                for m in range(12):
                    for k in range(KT):
                        nc.tensor.matmul(
                            gw[:, m, col],
                            whh_sb[:, k, m * 128:(m + 1) * 128],
                            h_prev[:, k, :],
                            start=False,
                            stop=(k == KT - 1),
                            skip_group_check=True,
                        )
                for m in range(12, MT):
                    for k in range(KT):
                        nc.tensor.matmul(
                            gwo[:, m - 12, col],
                            whh_sb[:, k, m * 128:(m + 1) * 128],
                            h_prev[:, k, :],
                            start=False,
                            stop=(k == KT - 1),
                            skip_group_check=True,
                        )
                sig = sgp.tile([128, 12, BC], F32, tag="sig")
                nc.scalar.activation(sig[:], gw[:, 0:12, col], AF.Sigmoid)
                so = sop.tile([128, 4, BC], F32, tag="sigo")
                nc.scalar.activation(so[:], gwo[:, 0:4, col], AF.Sigmoid)

                # c path: c_new = f*c + i*(2*sigmoid(2 z_g) - 1)
                fc = ewp.tile([128, KT, BC], F32, tag="fc")
                nc.vector.tensor_mul(fc[:], sig[:, 4:8, :], c_prev[:])
                t1 = ewp.tile([128, KT, BC], F32, tag="t1")
                nc.vector.scalar_tensor_tensor(
                    t1[:], sig[:, 8:12, :], 0.5, sig[:, 0:4, :],
                    ALU.subtract, ALU.mult,
                )
                c_new = cp.tile([128, KT, BC], F32, tag="c")
                nc.vector.scalar_tensor_tensor(
                    c_new[:], t1[:], 2.0, fc[:], ALU.mult, ALU.add
                )
                tct = ewp.tile([128, KT, BC], F32, tag="tct")
                nc.scalar.activation(tct[:], c_new[:], AF.Tanh)
                nc.vector.tensor_mul(hwt[:, s, :, :], so[:], tct[:])

                h_prev = hwt[:, s, :, :]
                c_prev = c_new

                # drip next window's prepass (17 chunks over WIN steps)
                if pending:
                    want = (s + 1) * 17 // WIN
                    while n_issued < want and pending:
                        pending.pop(0)()
                        n_issued += 1

                if s == WIN - 1:
                    nc.sync.dma_start(
                        hout_d[w], hwt[:].rearrange("p s k b -> p (s k b)")
                    )

    nc.compile()
    return nc


_NC_CACHE = {}


def _get_nc(T, variant=None):
    variant = variant or os.environ.get("BASS_LSTM_VARIANT", "v2")
    key = (T, variant)
    if key not in _NC_CACHE:
        _NC_CACHE[key] = _build_nc(T, variant)
    return _NC_CACHE[key]


_RUNNER_CACHE = {}


def _get_runner(nc):
    """Compile the SPMD executable once per program; reuse across calls.

    Forked from concourse.bass2jax.run_bass_via_pjrt (the @via_axon
    redirect target), minus the NTFF-trace path (unavailable here) and
    with the jitted callable cached so repeat kernel() calls skip the
    multi-minute walrus compile.
    """
    if id(nc) in _RUNNER_CACHE:
        return _RUNNER_CACHE[id(nc)]
    import jax
    from jax.sharding import Mesh, PartitionSpec
    from jax.experimental.shard_map import shard_map
    from concourse import bass2jax

    bass2jax.install_neuronx_cc_hook()

    partition_name = (
        nc.partition_id_tensor.name if nc.partition_id_tensor is not None else None
    )
    in_names, out_names, out_avals, zero_shapes = [], [], [], []
    for alloc in nc.m.functions[0].allocations:
        if not isinstance(alloc, mybir.MemoryLocationSet):
            continue
        name = alloc.memorylocations[0].name
        if alloc.kind == "ExternalInput":
            if name != partition_name:
                in_names.append(name)
        elif alloc.kind == "ExternalOutput":
            shape = tuple(alloc.tensor_shape)
            dtype = mybir.dt.np(alloc.dtype)
            out_names.append(name)
            out_avals.append(jax.core.ShapedArray(shape, dtype))
            zero_shapes.append((shape, dtype))
    n_params = len(in_names)
    all_in_names = in_names + out_names
    if partition_name is not None:
        all_in_names = all_in_names + [partition_name]

    def _body(*args):
        operands = list(args)
        if partition_name is not None:
            operands.append(bass2jax.partition_id_tensor())
        outs = bass2jax._bass_exec_p.bind(
            *operands,
            out_avals=tuple(out_avals),
            in_names=tuple(all_in_names),
            out_names=tuple(out_names),
            lowering_input_output_aliases=(),
            sim_require_finite=True,
            sim_require_nnan=True,
            nc=nc,
        )
        return tuple(outs)

    devices = jax.devices()[:NCORES]
    mesh = Mesh(np.asarray(devices), ("core",))
    nspecs = n_params + len(out_names)
    sharded = jax.jit(
        shard_map(
            _body,
            mesh=mesh,
            in_specs=(PartitionSpec("core"),) * nspecs,
            out_specs=(PartitionSpec("core"),) * len(out_names),
            check_rep=False,
        ),
        donate_argnums=tuple(range(n_params, nspecs)),
        keep_unused=True,
    )
    runner = (sharded, in_names, out_names, out_avals, zero_shapes)
    _RUNNER_CACHE[id(nc)] = runner
    return runner


def _run_spmd(nc, in_maps):
    sharded, in_names, out_names, out_avals, zero_shapes = _get_runner(nc)
    concat_in = [
        np.concatenate([np.asarray(in_maps[c][name]) for c in range(NCORES)], axis=0)
        for name in in_names
    ]
    concat_zeros = [
        np.zeros((NCORES * s[0], *s[1:]), dt) for (s, dt) in zero_shapes
    ]
    import time as _time

    t0 = _time.perf_counter()
    out_arrs = sharded(*concat_in, *concat_zeros)
    out_arrs = [np.asarray(a) for a in out_arrs]
    _run_spmd.last_wall_s = _time.perf_counter() - t0
    return [
        {
            name: out_arrs[i].reshape(NCORES, *out_avals[i].shape)[c]
            for i, name in enumerate(out_names)
        }
        for c in range(NCORES)
    ]


_run_spmd.last_wall_s = None


def _prep_core_inputs(x, lengths, wih, whh, wb, q, reverse, T):
    """Host-side input prep for one core (batch quarter q, one direction)."""
    xs = x[q * BC:(q + 1) * BC, :, :]  # [BC, T, D]
    ls = lengths[q * BC:(q + 1) * BC]  # [BC]
    if reverse:
        # per-example reversed-and-left-aligned: x_rev[b, s] = x[b, len-1-s]
        idx = ls[:, None] - 1 - np.arange(T)[None, :]  # [BC, T]
        valid = idx >= 0
        idx = np.maximum(idx, 0)
        xs = xs[np.arange(BC)[:, None], idx] * valid[:, :, None]
    xT = np.ascontiguousarray(xs.transpose(2, 1, 0).reshape(D, T * BC))
    ones = np.zeros((128, WIN * BC), np.float16)
    ones[0, :] = 1.0
    return {
        "xT": xT.astype(np.float16),
        "wih": wih,
        "whh": whh,
        "wb": wb,
        "ones": ones,
    }


def _prep_direction_weights(W_ih, W_hh, b_ih, b_hh):
    wih = np.ascontiguousarray(W_ih.T).astype(np.float32).copy()  # [D, G]
    whh = np.ascontiguousarray(W_hh.T).astype(np.float32).copy()  # [H, G]
    bsum = (b_ih + b_hh).astype(np.float32).copy()  # [G]
    # fold the tanh-via-sigmoid 2x into the g-gate block (cols 2H:3H)
    wih[:, 2 * H:3 * H] *= 2.0
    whh[:, 2 * H:3 * H] *= 2.0
    bsum[2 * H:3 * H] *= 2.0
    wb = np.zeros((128, G), np.float32)
    wb[0, :] = bsum
    return (
        wih.astype(np.float16),
        whh.astype(F8_NP),
        wb.astype(np.float16),
    )


def _scan_from_hout(hout, T):
    """hout [nwin, 128, WIN*KT*BC] -> scan output [BC, T, H] f32."""
    nwin = T // WIN
    a = np.asarray(hout).astype(np.float32).reshape(nwin, 128, WIN, KT, BC)
    # h[b, w*WIN+s, kt*128+p] = a[w, p, s, kt, b]
    return a.transpose(4, 0, 2, 3, 1).reshape(BC, T, H)


def kernel(x, lengths, W_ih_f, W_hh_f, b_ih_f, b_hh_f, W_ih_b, W_hh_b, b_ih_b, b_hh_b):
    T = x.shape[1]
    x = np.asarray(x, dtype=np.float32)
    lengths = np.asarray(lengths).astype(np.int64)

    wih_f, whh_f, wb_f = _prep_direction_weights(W_ih_f, W_hh_f, b_ih_f, b_hh_f)
    wih_b, whh_b, wb_b = _prep_direction_weights(W_ih_b, W_hh_b, b_ih_b, b_hh_b)

    in_maps = []
    for r in range(NCORES):
        reverse = r >= 4
        q = r % 4
        if reverse:
            m = _prep_core_inputs(x, lengths, wih_b, whh_b, wb_b, q, True, T)
        else:
            m = _prep_core_inputs(x, lengths, wih_f, whh_f, wb_f, q, False, T)
        in_maps.append(m)

    nc = _get_nc(T)
    results = _run_spmd(nc, in_maps)
    kernel.last_wall_s = _run_spmd.last_wall_s

    h_f = np.empty((B, T, H), np.float32)
    h_b = np.empty((B, T, H), np.float32)
    tidx = np.arange(T)
    for q in range(4):
        ls = lengths[q * BC:(q + 1) * BC]
        # forward: replace padded tail with h[len-1]
        hs = _scan_from_hout(results[q]["hout"], T)  # [BC, T, H]
        idx = np.minimum(tidx[None, :], (ls - 1)[:, None])  # [BC, T]
        h_f[q * BC:(q + 1) * BC] = hs[np.arange(BC)[:, None], idx]
        # backward: h_b[b, t] = h_scan[b, len-1-t] for t < len else 0
        hs = _scan_from_hout(results[q + 4]["hout"], T)
        idx = ls[:, None] - 1 - tidx[None, :]
        valid = idx >= 0
        idx = np.maximum(idx, 0)
        h_b[q * BC:(q + 1) * BC] = hs[np.arange(BC)[:, None], idx] * valid[:, :, None]

    return np.concatenate([h_f, h_b], axis=-1).astype(np.float32)


kernel.last_exec_time_ns = None
kernel.last_wall_s = None


# revision 13
# speedup vs baseline: 1.3059x; 1.0498x over previous
"""BiLSTM layer (B=32, T=512, D=512, H=512) as a Bass/Trainium2 kernel on 8 NeuronCores.

Sharding: 8 cores = 2 directions x 4 batch-quarters. Each core runs a full
forward LSTM scan over T=512 steps for 8 examples of one direction. The
backward direction is realized as a forward scan over per-example
reversed-and-left-aligned inputs prepared on the host:
    x_rev[b, s] = x[b, len_b-1-s]  for s < len_b, else 0.
With that alignment no masking is needed anywhere:
  - forward: the scan past len_b computes garbage, but the host replaces the
    padded tail with h[len-1] via a gather (the reference freezes h there).
  - backward: the scan over s < len_b is exactly the reference's reversed
    scan; outputs are re-scattered per example on the host, zeros elsewhere.

Gate pre-activations live directly in PSUM: a windowed prepass (W=16 steps)
computes xg = W_ih.T x + (b_ih + b_hh) into the per-step gate PSUM columns
(bias enters via a 5th matmul against a ones-row), and the per-step
recurrent matmuls accumulate W_hh.T h on top (start=False). This removes
the per-step DVE gate-add and all Act-engine prepass work.

The tanh for the g-gate is computed as 2*sigmoid(2z)-1 (the 2z folding is
pre-baked into the g rows/bias), so the i/f/g gates need one sigmoid per
step; tanh(c) and sigmoid(o) are separate Act ops.
"""

import os
import sys

import numpy as np

sys.path.insert(0, "/opt/trn_rl_repo")

import concourse.bass as bass  # noqa: E402
import concourse.bacc as bacc  # noqa: E402
import concourse.tile as tile  # noqa: E402
from concourse import mybir  # noqa: E402

import ml_dtypes  # noqa: E402

F32 = mybir.dt.float32
F16 = mybir.dt.float16
F8 = mybir.dt.float8e4  # e4m3 — used for W_hh only
F8_NP = mybir.dt.np(F8)
AF = mybir.ActivationFunctionType
ALU = mybir.AluOpType

B, D, H = 32, 512, 512
G = 4 * H  # 2048 gate channels
NCORES = 8
BC = 8  # batch per core
KT = D // 128  # 4 k-tiles
MT = G // 128  # 16 m-tiles
WIN = 16  # steps per gate-PSUM window (16 m-tiles * WIN*BC f32 = 8KB/partition)

_T_DEFAULT = 512


def _build_nc(T: int, variant: str = "v2"):
    """Build the SPMD single-core program (identical on all 8 cores)."""
    nwin = T // WIN
    nc = bacc.Bacc("TRN2", target_bir_lowering=False, debug=False, num_devices=NCORES)

    xT_d = nc.dram_tensor("xT", [D, T * BC], F16, kind="ExternalInput")
    wih_d = nc.dram_tensor("wih", [D, G], F16, kind="ExternalInput")
    whh_d = nc.dram_tensor("whh", [H, G], F8, kind="ExternalInput")
    wb_d = nc.dram_tensor("wb", [128, G], F16, kind="ExternalInput")
    ones_d = nc.dram_tensor("ones", [128, WIN * BC], F16, kind="ExternalInput")
    # outputs: c and sigmoid(o) windows; the host reconstructs h = so * tanh(c)
    cout_d = nc.dram_tensor(
        "cout", [nwin, 128, WIN * KT * BC], F32, kind="ExternalOutput"
    )
    oout_d = nc.dram_tensor(
        "oout", [nwin, 128, WIN * KT * BC], F16, kind="ExternalOutput"
    )

    with tile.TileContext(nc) as tc:
        with (
            tc.tile_pool(name="const", bufs=1) as constp,
            tc.tile_pool(name="xc", bufs=2) as xcp,
            tc.tile_pool(name="hw", bufs=2) as hp,
            tc.tile_pool(name="cw", bufs=2) as cwp,
            tc.tile_pool(name="ow", bufs=2) as owp,
            tc.tile_pool(name="sig", bufs=3) as sgp,
            tc.tile_pool(name="ew", bufs=3) as ewp,
            tc.tile_pool(name="gw", bufs=2, space="PSUM") as gp,
            tc.tile_pool(name="gwo", bufs=2, space="PSUM") as gop,
        ):
            # ---- persistent weights in SBUF ----
            wih_sb = constp.tile([128, KT, G], F16, tag="wih")
            whh_sb = constp.tile([128, KT, G], F8, tag="whh")
            wb_sb = constp.tile([128, G], F16, tag="wb")
            ones_sb = constp.tile([128, WIN * BC], F16, tag="ones")
            for k in range(KT):
                nc.sync.dma_start(wih_sb[:, k, :], wih_d[k * 128:(k + 1) * 128, :])
                nc.sync.dma_start(whh_sb[:, k, :], whh_d[k * 128:(k + 1) * 128, :])
            nc.sync.dma_start(wb_sb[:], wb_d[:])
            nc.sync.dma_start(ones_sb[:], ones_d[:])

            # ---- initial state ----
            h0 = constp.tile([128, KT, BC], F8, tag="h0")
            nc.vector.memset(h0[:], 0.0)
            c0 = constp.tile([128, KT, BC], F32, tag="c0")
            nc.vector.memset(c0[:], 0.0)

            gw_tiles = {}

            # ---- prepass: xg (+bias) for one window, directly into PSUM ----
            def make_prepass(w):
                # ifg gates (m 0..11) and o gates (m 12..15) in separate PSUM
                # tiles so the per-step o-gate matmuls don't pick up a false
                # WAR against sigma_ifg's read of the shared tile.
                gw = gp.tile([128, 12, WIN * BC], F32, tag="gw")
                gwo = gop.tile([128, 4, WIN * BC], F32, tag="gwo")
                xc = xcp.tile([128, KT, WIN * BC], F16, tag="xc")
                gw_tiles[w] = (gw, gwo)
                c0 = w * WIN * BC
                chunks = []

                def dma_in():
                    for k in range(KT):
                        nc.sync.dma_start(
                            xc[:, k, :], xT_d[k * 128:(k + 1) * 128, c0:c0 + WIN * BC]
                        )

                chunks.append(dma_in)

                def mtile(m):
                    # 4 m-slices (512B each) share a 2KB PSUM zero region; the
                    # first matmul into each region uses start=True (marks the
                    # whole region pending-zero), the rest write/accumulate.
                    dst = gw[:, m, :] if m < 12 else gwo[:, m - 12, :]
                    first = m % 4 == 0
                    for k in range(KT):
                        nc.tensor.matmul(
                            dst,
                            wih_sb[:, k, m * 128:(m + 1) * 128],
                            xc[:, k, :],
                            start=(first and k == 0),
                            stop=False,
                            skip_group_check=True,
                        )
                    # bias via ones-row: out[p, col] += wb[0, p] (row 0 of ones = 1)
                    nc.tensor.matmul(
                        dst,
                        wb_sb[:, m * 128:(m + 1) * 128],
                        ones_sb[:],
                        start=False,
                        stop=False,
                        skip_group_check=True,
                    )

                for m in range(MT):
                    chunks.append(lambda m=m: mtile(m))
                return chunks

            # prologue: window 0 fully
            for fn in make_prepass(0):
                fn()

            h_prev = h0[:]
            c_prev = c0[:]
            pending = []
            n_issued = 0
            hwt = cwt = owt = None
            DR = mybir.MatmulPerfMode.DoubleRow
            for t in range(T):
                w, s = divmod(t, WIN)
                if s == 0:
                    assert not pending, f"window {w}: {len(pending)} chunks undrained"
                    if w + 1 < nwin:
                        pending = make_prepass(w + 1)
                        n_issued = 0
                    hwt = hp.tile([128, WIN, KT, BC], F8, tag="hw")
                    cwt = cwp.tile([128, WIN, KT, BC], F32, tag="cw")
                    owt = owp.tile([128, WIN, KT, BC], F16, tag="ow")
                gw, gwo = gw_tiles[w]
                col = slice(s * BC, (s + 1) * BC)

                # recurrent matmuls (fp8 DoubleRow: 2 k-tiles per instruction)
                # accumulate onto the prepass xg columns
                for m in range(12):
                    for k in range(0, KT, 2):
                        nc.tensor.matmul(
                            gw[:, m, col],
                            whh_sb[:, k:k + 2, m * 128:(m + 1) * 128],
                            h_prev[:, k:k + 2, :],
                            start=False,
                            stop=(k == KT - 2),
                            perf_mode=DR,
                            skip_group_check=True,
                        )
                for m in range(12, MT):
                    for k in range(0, KT, 2):
                        nc.tensor.matmul(
                            gwo[:, m - 12, col],
                            whh_sb[:, k:k + 2, m * 128:(m + 1) * 128],
                            h_prev[:, k:k + 2, :],
                            start=False,
                            stop=(k == KT - 2),
                            perf_mode=DR,
                            skip_group_check=True,
                        )
                sig = sgp.tile([128, 12, BC], F32, tag="sig")
                nc.scalar.activation(sig[:], gw[:, 0:12, col], AF.Sigmoid)
                nc.scalar.activation(owt[:, s, :, :], gwo[:, 0:4, col], AF.Sigmoid)

                # c path: c_new = f*c + i*(2*sigmoid(2 z_g) - 1)
                fc = ewp.tile([128, KT, BC], F32, tag="fc")
                nc.vector.tensor_mul(fc[:], sig[:, 4:8, :], c_prev)
                t1 = ewp.tile([128, KT, BC], F32, tag="t1")
                nc.vector.scalar_tensor_tensor(
                    t1[:], sig[:, 8:12, :], 0.5, sig[:, 0:4, :],
                    ALU.subtract, ALU.mult,
                )
                nc.vector.scalar_tensor_tensor(
                    cwt[:, s, :, :], t1[:], 2.0, fc[:], ALU.mult, ALU.add
                )
                tct = ewp.tile([128, KT, BC], F32, tag="tct")
                nc.scalar.activation(tct[:], cwt[:, s, :, :], AF.Tanh)
                nc.vector.tensor_mul(hwt[:, s, :, :], owt[:, s, :, :], tct[:])

                h_prev = hwt[:, s, :, :]
                c_prev = cwt[:, s, :, :]

                # drip next window's prepass (17 chunks over WIN steps)
                if pending:
                    want = (s + 1) * 17 // WIN
                    while n_issued < want and pending:
                        pending.pop(0)()
                        n_issued += 1

                if s == WIN - 1:
                    nc.sync.dma_start(
                        cout_d[w], cwt[:].rearrange("p s k b -> p (s k b)")
                    )
                    nc.sync.dma_start(
                        oout_d[w], owt[:].rearrange("p s k b -> p (s k b)")
                    )

    nc.compile()
    return nc


_NC_CACHE = {}


def _get_nc(T, variant=None):
    variant = variant or os.environ.get("BASS_LSTM_VARIANT", "v2")
    key = (T, variant)
    if key not in _NC_CACHE:
        _NC_CACHE[key] = _build_nc(T, variant)
    return _NC_CACHE[key]


_RUNNER_CACHE = {}


def _get_runner(nc):
    """Compile the SPMD executable once per program; reuse across calls.

    Forked from concourse.bass2jax.run_bass_via_pjrt (the @via_axon
    redirect target), minus the NTFF-trace path (unavailable here) and
    with the jitted callable cached so repeat kernel() calls skip the
    multi-minute walrus compile.
    """
    if id(nc) in _RUNNER_CACHE:
        return _RUNNER_CACHE[id(nc)]
    import jax
    from jax.sharding import Mesh, PartitionSpec
    from jax.experimental.shard_map import shard_map
    from concourse import bass2jax

    bass2jax.install_neuronx_cc_hook()

    partition_name = (
        nc.partition_id_tensor.name if nc.partition_id_tensor is not None else None
    )
    in_names, out_names, out_avals, zero_shapes = [], [], [], []
    for alloc in nc.m.functions[0].allocations:
        if not isinstance(alloc, mybir.MemoryLocationSet):
            continue
        name = alloc.memorylocations[0].name
        if alloc.kind == "ExternalInput":
            if name != partition_name:
                in_names.append(name)
        elif alloc.kind == "ExternalOutput":
            shape = tuple(alloc.tensor_shape)
            dtype = mybir.dt.np(alloc.dtype)
            out_names.append(name)
            out_avals.append(jax.core.ShapedArray(shape, dtype))
            zero_shapes.append((shape, dtype))
    n_params = len(in_names)
    all_in_names = in_names + out_names
    if partition_name is not None:
        all_in_names = all_in_names + [partition_name]

    def _body(*args):
        operands = list(args)
        if partition_name is not None:
            operands.append(bass2jax.partition_id_tensor())
        outs = bass2jax._bass_exec_p.bind(
            *operands,
            out_avals=tuple(out_avals),
            in_names=tuple(all_in_names),
            out_names=tuple(out_names),
            lowering_input_output_aliases=(),
            sim_require_finite=True,
            sim_require_nnan=True,
            nc=nc,
        )
        return tuple(outs)

    devices = jax.devices()[:NCORES]
    mesh = Mesh(np.asarray(devices), ("core",))
    nspecs = n_params + len(out_names)
    sharded = jax.jit(
        shard_map(
            _body,
            mesh=mesh,
            in_specs=(PartitionSpec("core"),) * nspecs,
            out_specs=(PartitionSpec("core"),) * len(out_names),
            check_rep=False,
        ),
        donate_argnums=tuple(range(n_params, nspecs)),
        keep_unused=True,
    )
    runner = (sharded, in_names, out_names, out_avals, zero_shapes)
    _RUNNER_CACHE[id(nc)] = runner
    return runner


def _run_spmd(nc, in_maps):
    sharded, in_names, out_names, out_avals, zero_shapes = _get_runner(nc)
    concat_in = [
        np.concatenate([np.asarray(in_maps[c][name]) for c in range(NCORES)], axis=0)
        for name in in_names
    ]
    concat_zeros = [
        np.zeros((NCORES * s[0], *s[1:]), dt) for (s, dt) in zero_shapes
    ]
    import time as _time

    t0 = _time.perf_counter()
    out_arrs = sharded(*concat_in, *concat_zeros)
    out_arrs = [np.asarray(a) for a in out_arrs]
    _run_spmd.last_wall_s = _time.perf_counter() - t0
    return [
        {
            name: out_arrs[i].reshape(NCORES, *out_avals[i].shape)[c]
            for i, name in enumerate(out_names)
        }
        for c in range(NCORES)
    ]


_run_spmd.last_wall_s = None


def _prep_core_inputs(x, lengths, wih, whh, wb, q, reverse, T):
    """Host-side input prep for one core (batch quarter q, one direction)."""
    xs = x[q * BC:(q + 1) * BC, :, :]  # [BC, T, D]
    ls = lengths[q * BC:(q + 1) * BC]  # [BC]
    if reverse:
        # per-example reversed-and-left-aligned: x_rev[b, s] = x[b, len-1-s]
        idx = ls[:, None] - 1 - np.arange(T)[None, :]  # [BC, T]
        valid = idx >= 0
        idx = np.maximum(idx, 0)
        xs = xs[np.arange(BC)[:, None], idx] * valid[:, :, None]
    xT = np.ascontiguousarray(xs.transpose(2, 1, 0).reshape(D, T * BC))
    ones = np.zeros((128, WIN * BC), np.float16)
    ones[0, :] = 1.0
    return {
        "xT": xT.astype(np.float16),
        "wih": wih,
        "whh": whh,
        "wb": wb,
        "ones": ones,
    }


def _prep_direction_weights(W_ih, W_hh, b_ih, b_hh):
    wih = np.ascontiguousarray(W_ih.T).astype(np.float32).copy()  # [D, G]
    whh = np.ascontiguousarray(W_hh.T).astype(np.float32).copy()  # [H, G]
    bsum = (b_ih + b_hh).astype(np.float32).copy()  # [G]
    # fold the tanh-via-sigmoid 2x into the g-gate block (cols 2H:3H)
    wih[:, 2 * H:3 * H] *= 2.0
    whh[:, 2 * H:3 * H] *= 2.0
    bsum[2 * H:3 * H] *= 2.0
    wb = np.zeros((128, G), np.float32)
    wb[0, :] = bsum
    return (
        wih.astype(np.float16),
        whh.astype(F8_NP),
        wb.astype(np.float16),
    )


def _scan_from_out(res, T):
    """(cout, oout) [nwin, 128, WIN*KT*BC] -> scan output h [BC, T, H] f32."""
    nwin = T // WIN

    def to_bth(a):
        a = np.asarray(a).astype(np.float32).reshape(nwin, 128, WIN, KT, BC)
        # v[b, w*WIN+s, kt*128+p] = a[w, p, s, kt, b]
        return a.transpose(4, 0, 2, 3, 1).reshape(BC, T, H)

    return to_bth(res["oout"]) * np.tanh(to_bth(res["cout"]))


def kernel(x, lengths, W_ih_f, W_hh_f, b_ih_f, b_hh_f, W_ih_b, W_hh_b, b_ih_b, b_hh_b):
    T = x.shape[1]
    x = np.asarray(x, dtype=np.float32)
    lengths = np.asarray(lengths).astype(np.int64)

    wih_f, whh_f, wb_f = _prep_direction_weights(W_ih_f, W_hh_f, b_ih_f, b_hh_f)
    wih_b, whh_b, wb_b = _prep_direction_weights(W_ih_b, W_hh_b, b_ih_b, b_hh_b)

    in_maps = []
    for r in range(NCORES):
        reverse = r >= 4
        q = r % 4
        if reverse:
            m = _prep_core_inputs(x, lengths, wih_b, whh_b, wb_b, q, True, T)
        else:
            m = _prep_core_inputs(x, lengths, wih_f, whh_f, wb_f, q, False, T)
        in_maps.append(m)

    nc = _get_nc(T)
    results = _run_spmd(nc, in_maps)
    kernel.last_wall_s = _run_spmd.last_wall_s

    h_f = np.empty((B, T, H), np.float32)
    h_b = np.empty((B, T, H), np.float32)
    tidx = np.arange(T)
    for q in range(4):
        ls = lengths[q * BC:(q + 1) * BC]
        # forward: replace padded tail with h[len-1]
        hs = _scan_from_out(results[q], T)  # [BC, T, H]
        idx = np.minimum(tidx[None, :], (ls - 1)[:, None])  # [BC, T]
        h_f[q * BC:(q + 1) * BC] = hs[np.arange(BC)[:, None], idx]
        # backward: h_b[b, t] = h_scan[b, len-1-t] for t < len else 0
        hs = _scan_from_out(results[q + 4], T)
        idx = ls[:, None] - 1 - tidx[None, :]
        valid = idx >= 0
        idx = np.maximum(idx, 0)
        h_b[q * BC:(q + 1) * BC] = hs[np.arange(BC)[:, None], idx] * valid[:, :, None]

    return np.concatenate([h_f, h_b], axis=-1).astype(np.float32)


kernel.last_exec_time_ns = None
kernel.last_wall_s = None
